# revision 7
# baseline (speedup 1.0000x reference)
"""ABAE pooling kernel for 8 TRN2 NeuronCores.

Sharding: data-parallel over the batch dim (512 -> 64 per core); the
embedding table E_w and the small weights are replicated.

Math notes (all exploit l2norm scale invariance):
  - a_i's global exp-sum normalization cancels inside l2norm(z_s).
  - the softmax denominator cancels inside l2norm(p_t @ T_w).
  - the 1/L mean cancels inside l2norm(z_n).
So every batch row is fully independent -> no collectives.

Per-core device program (B=64 local batches, L=100, d=256, M=10 negs):
  pos:  gather 6400 rows of E_w -> e tiles [128,256] x50 (resident in SBUF)
        y_sum   = blockdiag-mask.T @ e            (PE, accumulate 50 mm)
        My      = (y_sum @ M_w.T)/L + M_b         (PE, via transposed y_sum)
        My_exp  = maskT.T @ My  (row-broadcast)   (PE, per tile)
        d_i     = rowsum(e * My_exp)              (DVE tensor_tensor_reduce)
        w       = exp(tanh(d_i))                  (ACT)
        z_pre   = (mask*w).T @ e                  (PE, accumulate 50 mm)
        z_s     = l2norm(z_pre)                   (DVE+ACT)
        p_preT  = lin_w @ z_s.T + lin_b           (PE, via transposed z_s)
        r_s     = l2norm(exp(p_preT).T @ T_w)     (PE+DVE+ACT)
  negs: 10 chunks x 50 gathered tiles, pooled by the same blockdiag mask
        into [64,256] psum accumulators, then l2norm -> z_n rows.
"""

import numpy as np

import concourse.bass as bass
import concourse.mybir as mybir
import concourse.tile as tile
from concourse import bacc
from concourse.bass_utils import run_bass_kernel_spmd
from concourse.masks import make_identity

P = 128
D = 256
L = 100
B_LOC = 64          # batches per core
N_CORES = 8
N_ASP = 14
M_NEG = 10
POS_TILES = B_LOC * L // P          # 50
NEG_CHUNKS = M_NEG                  # 10 chunks of 64 groups each
CHUNK_TILES = B_LOC * L // P        # 50 tiles per chunk
VOCAB = 100000

F32 = mybir.dt.float32
I32 = mybir.dt.int32


def _build_graph():
    nc = bacc.Bacc(None, target_bir_lowering=False)

    # ---- DRAM parameters (per-core shards / replicated weights) ----
    ew = nc.declare_dram_parameter("E_w", [VOCAB, D], F32, isOutput=False)
    idxp_d = nc.declare_dram_parameter("idxp", [P, POS_TILES], I32, isOutput=False)
    idxn_d = nc.declare_dram_parameter(
        "idxn", [P, NEG_CHUNKS * CHUNK_TILES], I32, isOutput=False
    )
    mask_d = nc.declare_dram_parameter(
        "mask_sb", [P, POS_TILES * B_LOC], F32, isOutput=False
    )
    maskT_d = nc.declare_dram_parameter(
        "maskT_sb", [B_LOC, POS_TILES * P], F32, isOutput=False
    )
    mw_d = nc.declare_dram_parameter("mw_rhs", [P, 2 * D], F32, isOutput=False)
    mb_d = nc.declare_dram_parameter("mb_row", [1, D], F32, isOutput=False)
    linw_d = nc.declare_dram_parameter("linw_sb", [P, 2 * N_ASP], F32, isOutput=False)
    linb_d = nc.declare_dram_parameter("linb_row", [1, N_ASP], F32, isOutput=False)
    tw_d = nc.declare_dram_parameter("tw_sb", [N_ASP, D], F32, isOutput=False)

    rs_out = nc.declare_dram_parameter("r_s_out", [B_LOC, D], F32, isOutput=True)
    zs_out = nc.declare_dram_parameter("z_s_out", [B_LOC, D], F32, isOutput=True)
    zn_out = nc.declare_dram_parameter(
        "z_n_out", [NEG_CHUNKS * B_LOC, D], F32, isOutput=True
    )

    with tile.TileContext(nc) as tc:
        with (
            tc.tile_pool(name="const", bufs=1) as cpool,
            tc.tile_pool(name="epos", bufs=POS_TILES) as epool,
            tc.tile_pool(name="en", bufs=64) as enpool,
            tc.tile_pool(name="work", bufs=2) as wpool,
            tc.tile_pool(name="outs", bufs=2) as opool,
            tc.tile_pool(name="acc", bufs=2, space="PSUM") as acc_pool,
            tc.tile_pool(name="myexp", bufs=2, space="PSUM") as me_pool,
            tc.tile_pool(name="znacc", bufs=2, space="PSUM") as zn_pool,
            tc.tile_pool(name="psmall", bufs=2, space="PSUM") as ps_pool,
        ):
            # ---- load constants ----
            idxp = cpool.tile([P, POS_TILES], I32, tag="idxp")
            idxn = cpool.tile([P, NEG_CHUNKS * CHUNK_TILES], I32, tag="idxn")
            mask = cpool.tile([P, POS_TILES * B_LOC], F32, tag="mask")
            maskT = cpool.tile([B_LOC, POS_TILES * P], F32, tag="maskT")
            mw = cpool.tile([P, 2 * D], F32, tag="mw")
            mb = cpool.tile([1, D], F32, tag="mb")
            linw = cpool.tile([P, 2 * N_ASP], F32, tag="linw")
            linb = cpool.tile([1, N_ASP], F32, tag="linb")
            tw = cpool.tile([N_ASP, D], F32, tag="tw")
            ident = cpool.tile([P, P], F32, tag="ident")
            ones = cpool.tile([1, B_LOC], F32, tag="ones")

            nc.sync.dma_start(out=idxp[:], in_=idxp_d[:])
            nc.sync.dma_start(out=idxn[:], in_=idxn_d[:])
            nc.sync.dma_start(out=mask[:], in_=mask_d[:])
            nc.sync.dma_start(out=maskT[:], in_=maskT_d[:])
            nc.sync.dma_start(out=mw[:], in_=mw_d[:])
            nc.sync.dma_start(out=mb[:], in_=mb_d[:])
            nc.sync.dma_start(out=linw[:], in_=linw_d[:])
            nc.sync.dma_start(out=linb[:], in_=linb_d[:])
            nc.sync.dma_start(out=tw[:], in_=tw_d[:])
            make_identity(nc, ident[:])
            nc.gpsimd.memset(ones[:], 1.0)

            def l2norm_rows(psum_ap, out_sb, n_rows):
                """out_sb = psum_ap / ||psum_ap||_2 (row-wise).

                DVE may read at most one input from PSUM, so stage
                through an SBUF copy first.
                """
                raw = wpool.tile([n_rows, D], F32, tag="l2raw")
                sq = wpool.tile([n_rows, D], F32, tag="sq_scr")
                ss = wpool.tile([n_rows, 1], F32, tag="ss")
                nrm = wpool.tile([n_rows, 1], F32, tag="nrm")
                inv = wpool.tile([n_rows, 1], F32, tag="inv")
                nc.vector.tensor_copy(out=raw[:], in_=psum_ap)
                nc.vector.tensor_tensor(
                    out=sq[:], in0=raw[:], in1=raw[:], op=mybir.AluOpType.mult
                )
                nc.vector.reduce_sum(
                    out=ss[:], in_=sq[:], axis=mybir.AxisListType.X
                )
                nc.scalar.activation(nrm[:], ss[:], mybir.ActivationFunctionType.Sqrt)
                nc.vector.reciprocal(inv[:], nrm[:])
                nc.vector.tensor_scalar_mul(out_sb, in0=raw[:], scalar1=inv[:, 0:1])

            # ---- pos: gather ----
            ep = []
            for t in range(POS_TILES):
                e_t = epool.tile([P, D], F32, tag="epos")
                nc.gpsimd.indirect_dma_start(
                    out=e_t[:], out_offset=None,
                    in_=ew[:],
                    in_offset=bass.IndirectOffsetOnAxis(
                        ap=idxp[:, t : t + 1], axis=0
                    ),
                )
                ep.append(e_t)

            # ---- y_sum = sum over tokens ----
            ysum = acc_pool.tile([B_LOC, D], F32, tag="acc")
            for t in range(POS_TILES):
                nc.tensor.matmul(
                    out=ysum[:],
                    lhsT=mask[:, t * B_LOC : (t + 1) * B_LOC],
                    rhs=ep[t][:],
                    start=(t == 0), stop=(t == POS_TILES - 1),
                )
            ysum_sb = wpool.tile([B_LOC, D], F32, tag="ysum_sb")
            nc.vector.tensor_copy(out=ysum_sb[:], in_=ysum[:])

            # transpose y_sum -> [256, 64] as two [128, 64] tiles
            ysumT = []
            for h in range(2):
                tp = ps_pool.tile([P, B_LOC], F32, tag="psmall")
                nc.tensor.transpose(
                    out=tp[:],
                    in_=ysum_sb[:, h * P : (h + 1) * P],
                    identity=ident[:B_LOC, :B_LOC],
                )
                tsb = wpool.tile([P, B_LOC], F32, tag=f"ysumT{h}")
                nc.vector.tensor_copy(out=tsb[:], in_=tp[:])
                ysumT.append(tsb)

            # My = y_sum @ (M_w.T / L) + M_b  -> [64, 256]
            my_ps = acc_pool.tile([B_LOC, D], F32, tag="acc")
            nc.tensor.matmul(
                out=my_ps[:], lhsT=ysumT[0][:], rhs=mw[:, 0:D], start=True, stop=False
            )
            nc.tensor.matmul(
                out=my_ps[:], lhsT=ysumT[1][:], rhs=mw[:, D : 2 * D],
                start=False, stop=False,
            )
            nc.tensor.matmul(
                out=my_ps[:], lhsT=ones[:], rhs=mb[:], start=False, stop=True
            )
            my_sb = wpool.tile([B_LOC, D], F32, tag="my_sb")
            nc.vector.tensor_copy(out=my_sb[:], in_=my_ps[:])

            # d_i = rowsum(e * My_exp); then exp(tanh(.))
            dcol = wpool.tile([P, POS_TILES], F32, tag="dcol")
            for t in range(POS_TILES):
                mexp = me_pool.tile([P, D], F32, tag="myexp")
                nc.tensor.matmul(
                    out=mexp[:],
                    lhsT=maskT[:, t * P : (t + 1) * P],
                    rhs=my_sb[:],
                    start=True, stop=True,
                )
                scr = wpool.tile([P, D], F32, tag="ttr_scr")
                nc.vector.tensor_tensor(
                    out=scr[:], in0=ep[t][:], in1=mexp[:],
                    op=mybir.AluOpType.mult,
                )
                nc.vector.reduce_sum(
                    out=dcol[:, t : t + 1], in_=scr[:], axis=mybir.AxisListType.X
                )
            dtanh = wpool.tile([P, POS_TILES], F32, tag="dtanh")
            dexp = wpool.tile([P, POS_TILES], F32, tag="dexp")
            nc.scalar.activation(
                dtanh[:], dcol[:], mybir.ActivationFunctionType.Tanh
            )
            nc.scalar.activation(
                dexp[:], dtanh[:], mybir.ActivationFunctionType.Exp
            )

            # z_pre = sum_l w[l] * e[l]
            zpre = acc_pool.tile([B_LOC, D], F32, tag="acc")
            for t in range(POS_TILES):
                wt = wpool.tile([P, B_LOC], F32, tag="wtile")
                nc.vector.tensor_scalar_mul(
                    out=wt[:],
                    in0=mask[:, t * B_LOC : (t + 1) * B_LOC],
                    scalar1=dexp[:, t : t + 1],
                )
                nc.tensor.matmul(
                    out=zpre[:], lhsT=wt[:], rhs=ep[t][:],
                    start=(t == 0), stop=(t == POS_TILES - 1),
                )
            zs_sb = opool.tile([B_LOC, D], F32, tag="zs_sb")
            l2norm_rows(zpre[:], zs_sb[:], B_LOC)
            nc.sync.dma_start(out=zs_out[:], in_=zs_sb[:])

            # transpose z_s -> two [128, 64] tiles
            zsT = []
            for h in range(2):
                tp = ps_pool.tile([P, B_LOC], F32, tag="psmall")
                nc.tensor.transpose(
                    out=tp[:],
                    in_=zs_sb[:, h * P : (h + 1) * P],
                    identity=ident[:B_LOC, :B_LOC],
                )
                tsb = wpool.tile([P, B_LOC], F32, tag=f"zsT{h}")
                nc.vector.tensor_copy(out=tsb[:], in_=tp[:])
                zsT.append(tsb)

            # p_preT = lin_w @ z_s.T + lin_b  -> [14, 64]
            ppreT = ps_pool.tile([N_ASP, B_LOC], F32, tag="psmall")
            nc.tensor.matmul(
                out=ppreT[:], lhsT=linw[:, 0:N_ASP], rhs=zsT[0][:],
                start=True, stop=False,
            )
            nc.tensor.matmul(
                out=ppreT[:], lhsT=linw[:, N_ASP:], rhs=zsT[1][:],
                start=False, stop=False,
            )
            nc.tensor.matmul(
                out=ppreT[:], lhsT=linb[:], rhs=ones[:], start=False, stop=True
            )
            expp = wpool.tile([N_ASP, B_LOC], F32, tag="expp")
            nc.scalar.activation(
                expp[:], ppreT[:], mybir.ActivationFunctionType.Exp
            )

            # r_pre = exp(p_pre) @ T_w  -> [64, 256]
            rpre = ps_pool.tile([B_LOC, D], F32, tag="psmall")
            nc.tensor.matmul(
                out=rpre[:], lhsT=expp[:], rhs=tw[:], start=True, stop=True
            )
            rs_sb = opool.tile([B_LOC, D], F32, tag="rs_sb")
            l2norm_rows(rpre[:], rs_sb[:], B_LOC)
            nc.sync.dma_start(out=rs_out[:], in_=rs_sb[:])

            # ---- negs: 10 chunks of 64 groups ----
            for c in range(NEG_CHUNKS):
                znp = zn_pool.tile([B_LOC, D], F32, tag="znacc")
                for t in range(CHUNK_TILES):
                    en = enpool.tile([P, D], F32, tag="en")
                    nc.gpsimd.indirect_dma_start(
                        out=en[:], out_offset=None,
                        in_=ew[:],
                        in_offset=bass.IndirectOffsetOnAxis(
                            ap=idxn[:, c * CHUNK_TILES + t : c * CHUNK_TILES + t + 1],
                            axis=0,
                        ),
                    )
                    nc.tensor.matmul(
                        out=znp[:],
                        lhsT=mask[:, t * B_LOC : (t + 1) * B_LOC],
                        rhs=en[:],
                        start=(t == 0), stop=(t == CHUNK_TILES - 1),
                    )
                zn_sb = opool.tile([B_LOC, D], F32, tag="zn_sb")
                l2norm_rows(znp[:], zn_sb[:], B_LOC)
                nc.sync.dma_start(
                    out=zn_out[c * B_LOC : (c + 1) * B_LOC, :], in_=zn_sb[:]
                )

    nc.finalize()
    return nc


_GRAPH_CACHE = {}


def _get_graph():
    if "nc" not in _GRAPH_CACHE:
        _GRAPH_CACHE["nc"] = _build_graph()
    return _GRAPH_CACHE["nc"]


def _sbuf_idx_layout(flat_idx: np.ndarray) -> np.ndarray:
    """[T*128] row indices -> [128, T] int32, column t = indices of tile t."""
    t = flat_idx.size // P
    return np.ascontiguousarray(flat_idx.reshape(t, P).T.astype(np.int32))


def _make_masks():
    # mask_sb[p, t*64 + b] = 1 if (t*128+p)//100 == b
    r = np.arange(POS_TILES * P)
    g = r // L  # group id 0..63
    mask = np.zeros((POS_TILES * P, B_LOC), np.float32)
    mask[r, g] = 1.0
    mask3 = mask.reshape(POS_TILES, P, B_LOC)
    mask_sb = np.ascontiguousarray(
        mask3.transpose(1, 0, 2).reshape(P, POS_TILES * B_LOC)
    )
    maskT_sb = np.ascontiguousarray(
        mask3.transpose(2, 0, 1).reshape(B_LOC, POS_TILES * P)
    )
    return mask_sb, maskT_sb


def kernel(pos, negs, E_w, T_w, M_w, M_b, lin_w, lin_b, _trace=False, _trace_kwargs=None):
    pos = np.asarray(pos)
    negs = np.asarray(negs)
    E_w = np.ascontiguousarray(np.asarray(E_w, np.float32))
    T_w = np.asarray(T_w, np.float32)
    M_w = np.asarray(M_w, np.float32)
    M_b = np.asarray(M_b, np.float32)
    lin_w = np.asarray(lin_w, np.float32)
    lin_b = np.asarray(lin_b, np.float32)

    mask_sb, maskT_sb = _make_masks()
    mw_rhs = np.ascontiguousarray(
        (M_w.T / float(L)).reshape(2, P, D).transpose(1, 0, 2).reshape(P, 2 * D)
    ).astype(np.float32)
    mb_row = np.ascontiguousarray(M_b[None, :])
    linw_sb = np.ascontiguousarray(
        lin_w.T.reshape(2, P, N_ASP).transpose(1, 0, 2).reshape(P, 2 * N_ASP)
    ).astype(np.float32)
    linb_row = np.ascontiguousarray(lin_b[None, :])
    tw_sb = np.ascontiguousarray(T_w)

    shared = {
        "E_w": E_w,
        "mask_sb": mask_sb,
        "maskT_sb": maskT_sb,
        "mw_rhs": mw_rhs,
        "mb_row": mb_row,
        "linw_sb": linw_sb,
        "linb_row": linb_row,
        "tw_sb": tw_sb,
    }
    in_maps = []
    for k in range(N_CORES):
        b0 = k * B_LOC
        in_maps.append(
            dict(
                shared,
                idxp=_sbuf_idx_layout(np.asarray(pos[b0 : b0 + B_LOC]).reshape(-1)),
                idxn=_sbuf_idx_layout(
                    np.asarray(negs[b0 : b0 + B_LOC]).reshape(-1)
                ),
            )
        )

    nc = _get_graph()
    res = run_bass_kernel_spmd(
        nc,
        in_maps,
        core_ids=list(range(N_CORES)),
        trace=_trace,
        **(_trace_kwargs or {}),
    )
    outs = res.results
    r_s = np.concatenate([outs[k]["r_s_out"] for k in range(N_CORES)], axis=0)
    z_s = np.concatenate([outs[k]["z_s_out"] for k in range(N_CORES)], axis=0)
    z_n = np.concatenate(
        [outs[k]["z_n_out"].reshape(B_LOC, M_NEG, D) for k in range(N_CORES)],
        axis=0,
    )
    if _trace:
        return (r_s, z_s, z_n), res
    return (r_s, z_s, z_n)
